# revision 3
# baseline (speedup 1.0000x reference)
"""Causal MHA (B=4, T=2048, E=1024, H=16, D=64) on 8 TRN2 cores — bf16,
software-pipelined.

Sharding: core c = (batch b = c//2, head-half h = c%2); 8 heads per core as
2 quads of 4 (pairs s2). All matmuls bf16 (fp32 PSUM): fp8 was measured to
break the 2e-2 accuracy gate (3.6% multiplicative noise per quantized
element lands ~2.5e-2 on the output for every fp8-touched path).

Structure per (quad, pair s2, query-group g of 256, chunk-pair cp):
  - scores S^T[key, query] per head via K=64 matmuls (k/q with d on
    partitions, 2 heads per pair at partition 0/64), into a [128, 1024]
    PSUM tile = 2 heads x 2 chunks x 256 queries;
  - one wide exp per unit (act engine, scale=0.125, no max-subtraction);
  - AV with the ones-block trick: va = [128 key, 4 head, 64 v | 64 ones]
    so numerators and denominators come from the same matmul.

Schedule: the PE is the bottleneck (~225us of matmul); exp (~150us), DVE
(~120us) hide under it. The unit stream emits QK(u+1) then ~1us of pumped
background PE work (next-quad projection pieces, output-projection stages)
then AV(u), so the PE never stalls on exp. Diagonal units tri-mask the exp
output on DVE; normalize copies PSUM out early (DVE) to free the po bank.
Attention outputs are exchanged per-pair via AllGather as soon as columns
are final; the last pair's tail is split fine and stage-2 of the output
projection chases the exchanges so only a small collective ends the
critical path.
"""

import numpy as np

_B, _T, _E, _H, _D = 4, 2048, 1024, 16, 64
_NCORES = 8
_MYE = _E // 2

_EXP_SCALE = 0.125


def _build_nc(repeats=1, local_cc=False, debug=False):
    import concourse.mybir as mybir
    import concourse.tile as tile
    from concourse import bacc

    f32 = mybir.dt.float32
    bf16 = mybir.dt.bfloat16
    EXP = mybir.ActivationFunctionType.Exp

    nc = bacc.Bacc("TRN2", target_bir_lowering=False, debug=False,
                   num_devices=_NCORES)

    # host-prearranged layouts; flat [128, free] so one contiguous DMA
    # fills each SBUF tile
    xt_d = nc.dram_tensor("xt", [128, 8 * _T], bf16, kind="ExternalInput").ap()
    wk_d = nc.dram_tensor("wk", [128, 4096], bf16, kind="ExternalInput").ap()
    wq_d = nc.dram_tensor("wq", [128, 4096], bf16, kind="ExternalInput").ap()
    wv_d = nc.dram_tensor("wv", [128, 4096], bf16, kind="ExternalInput").ap()
    wot_d = nc.dram_tensor("wot", [128, 4096], bf16,
                           kind="ExternalInput").ap()
    bo_d = nc.dram_tensor("bo_b", [128, _MYE], f32, kind="ExternalInput").ap()
    tri_d = nc.dram_tensor("tri", [128, 128], bf16, kind="ExternalInput").ap()
    y_d = nc.dram_tensor("y", [_T, _MYE], f32, kind="ExternalOutput").ap()
    if debug:
        dbg_d = {nm: nc.dram_tensor(f"dbg_{nm}", shp, f32,
                                    kind="ExternalOutput").ap()
                 for nm, shp in [("kt", [128, _T]), ("qt", [128, _T]),
                                 ("va", [128, 512]), ("attg", [128, 2 * _T]),
                                 ("ysbA", [128, 512]),
                                 ("att", [128, _T]), ("pn", [64, 256]),
                                 ("rec", [64, 256])]}

    xt_src = xt_d.rearrange("p (e t) -> p e t", e=8)

    with tile.TileContext(nc) as tc:
        with (
            tc.tile_pool(name="big", bufs=1) as big,
            tc.tile_pool(name="wpool", bufs=2) as wpool,
            tc.tile_pool(name="ptp", bufs=4) as ptp,
            tc.tile_pool(name="nrm", bufs=3) as nrm,
            tc.tile_pool(name="sml", bufs=2) as sml,
            tc.tile_pool(name="ysb", bufs=2) as ysbp,
            tc.tile_pool(name="dram", bufs=1, space="DRAM") as dram,
            tc.tile_pool(name="ps", bufs=2, space="PSUM") as ps,
            tc.tile_pool(name="psc", bufs=2, space="PSUM") as psc,
            tc.tile_pool(name="ppo", bufs=1, space="PSUM") as ppo,
        ):
          for _rep in range(repeats):
            xt = big.tile([128, 8, _T], bf16, tag="xt")

            _xt_blocks = [(0, 256), (256, 256), (512, 512),
                          (1024, 512), (1536, 512)]

            def xt_dma(cb):
                lo, n = _xt_blocks[cb]
                nc.sync.dma_start(xt[:, :, lo:lo + n],
                                  xt_src[:, :, lo:lo + n])

            def load_quad_weights(qd, first=False):
                # wk/wq: [128p, 2s2, 8e, 128(2h x 64d)]; wv: [128p, 8e, 256]
                wk_t = wpool.tile([128, 2, 8, 128], bf16, tag="wk",
                                  name=f"wk{qd}")
                wq_t = wpool.tile([128, 2, 8, 128], bf16, tag="wq",
                                  name=f"wq{qd}")
                wv_t = wpool.tile([128, 8, 256], bf16, tag="wv",
                                  name=f"wv{qd}")
                base = 2048 * qd
                # s2=0 halves first: the prologue's first pieces need them
                nc.sync.dma_start(wk_t[:, 0], wk_d[:, base:base + 1024])
                if first:
                    xt_dma(0)
                nc.sync.dma_start(wq_t[:, 0], wq_d[:, base:base + 1024])
                if first:
                    xt_dma(1)
                nc.sync.dma_start(wk_t[:, 1], wk_d[:, base + 1024:base + 2048])
                nc.sync.dma_start(wq_t[:, 1], wq_d[:, base + 1024:base + 2048])
                nc.sync.dma_start(wv_t[:], wv_d[:, base:base + 2048])
                if first:
                    xt_dma(2)
                    xt_dma(3)
                    xt_dma(4)
                return wk_t, wq_t, wv_t

            wtiles = [None, None]
            wtiles[0] = load_quad_weights(0, first=True)

            tri_t = big.tile([128, 128], bf16, tag="tri")
            nc.sync.dma_start(tri_t[:], tri_d)
            tri_b = tri_t[:].rearrange("p (one m) -> p one m", one=1) \
                            .to_broadcast((128, 2, 128))

            wot_t = big.tile([128, 4, 2, _MYE], bf16, tag="wot")
            nc.sync.dma_start(wot_t[:], wot_d)
            bo_t = big.tile([128, _MYE], f32, tag="bo")
            nc.sync.dma_start(bo_t[:], bo_d)
            warm_t = big.tile([128, 1], f32, tag="warm")
            nc.gpsimd.memset(warm_t[:], 0.0)
            # warm the act engine's exp table during the prologue DMAs
            nc.scalar.activation(warm_t[:], warm_t[:], EXP)
            # ramp the PE clock to full p-state while DMAs land
            wjunk = big.tile([128, 128], bf16, tag="wjunk")
            nc.gpsimd.memset(wjunk[:], 0.0)
            pwarm = ps.tile([128, 512], f32, tag="proj")
            for _w in range(56):
                nc.tensor.matmul(pwarm[:, 0:128], wjunk[:], wjunk[:],
                                 start=(_w == 0), stop=(_w == 55))

            # gathered attention (pair j, slot s = global head-half s)
            attg = [big.tile([128, 2, _T], bf16, tag=f"attg{j}",
                             name=f"attg{j}")
                    for j in range(4)]
            ysbA = [big.tile([128, 512], bf16, tag=f"ysbA{tb}",
                             name=f"ysbA{tb}")
                    for tb in range(16)]

            # k^T/q^T per (quad, pair): [128 (2h x 64d), T]
            ktq = [[big.tile([128, _T], bf16, tag=f"kt{qd}_{s2}",
                             name=f"kt{qd}_{s2}") for s2 in (0, 1)]
                   for qd in (0, 1)]
            qtq = [[big.tile([128, _T], bf16, tag=f"qt{qd}_{s2}",
                             name=f"qt{qd}_{s2}") for s2 in (0, 1)]
                   for qd in (0, 1)]
            # v per (quad, key chunk): [128 key, 4h, 64 v | 64 ones]
            va = [[big.tile([128, 4, 128], bf16, tag=f"va{qd}_{c}",
                            name=f"va{qd}_{c}")
                   for c in range(16)] for qd in (0, 1)]

            # ---- projection pieces (bg-pumpable PE work), 256-col grain ----
            emitted = set()

            def kq_piece(qd, which, s2, half):
                key = (qd, which, s2, half)
                if key in emitted:
                    return False
                emitted.add(key)
                wk_t, wq_t, _ = wtiles[qd]
                wt, dst = ((wk_t, ktq[qd][s2]) if which == "k"
                           else (wq_t, qtq[qd][s2]))
                cols = slice(256 * half, 256 * half + 256)
                pk = ps.tile([128, 512], f32, tag="proj")
                for e in range(8):
                    nc.tensor.matmul(pk[:, 0:256], wt[:, s2, e, :],
                                     xt[:, e, cols],
                                     start=(e == 0), stop=(e == 7))
                nc.vector.tensor_copy(dst[:, cols], pk[:, 0:256])
                return True

            def v_piece(qd, c):
                key = (qd, "v", c)
                if key in emitted:
                    return False
                emitted.add(key)
                _, _, wv_t = wtiles[qd]
                pv = ps.tile([128, 512], f32, tag="proj")
                for e in range(8):
                    nc.tensor.matmul(
                        pv[:, 0:256], xt[:, e, 128 * c:128 * c + 128],
                        wv_t[:, e], start=(e == 0), stop=(e == 7))
                src = pv[:, 0:256].rearrange("p (h x) -> p h x", h=4)
                nc.vector.tensor_copy(va[qd][c][:, :, 0:64], src[:])
                return True

            def ones_piece(qd, c):
                key = (qd, "ones", c)
                if key in emitted:
                    return False
                emitted.add(key)
                nc.gpsimd.memset(va[qd][c][:, :, 64:128], 1.0)
                return True

            bg = []

            def pump(n=1):
                done = 0
                while bg and done < n:
                    if bg.pop(0)() is not False:
                        done += 1

            def queue_quad_bg(qd):
                for half in range(8):
                    for s2 in (0, 1):
                        bg.append(lambda qd=qd, s2=s2, half=half:
                                  kq_piece(qd, "k", s2, half))
                        bg.append(lambda qd=qd, s2=s2, half=half:
                                  kq_piece(qd, "q", s2, half))
                    bg.append(lambda qd=qd, c=2 * half: v_piece(qd, c))
                    bg.append(lambda qd=qd, c=2 * half + 1: v_piece(qd, c))
                    bg.append(lambda qd=qd, c=2 * half: ones_piece(qd, c))
                    bg.append(lambda qd=qd, c=2 * half + 1: ones_piece(qd, c))

            # quad-0 prologue: just enough for the first attention unit
            kq_piece(0, "k", 0, 0)
            kq_piece(0, "q", 0, 0)
            v_piece(0, 0)
            v_piece(0, 1)
            ones_piece(0, 0)
            ones_piece(0, 1)
            queue_quad_bg(0)
            wtiles[1] = load_quad_weights(1)
            queue_quad_bg(1)

            # ---- output projection ----
            def stage1(tb):
                py = ps.tile([128, 512], f32, tag="proj")
                n = 0
                for j in (0, 1):
                    for s in (0, 1):
                        nc.tensor.matmul(
                            py[:], attg[j][:, s, 128 * tb:128 * tb + 128],
                            wot_t[:, j, s, :], start=(n == 0), stop=(n == 3))
                        n += 1
                nc.vector.tensor_add(ysbA[tb][:], py[:], bo_t[:])

            def stage2(tb):
                py = ps.tile([128, 512], f32, tag="proj")
                n = 0
                for j in (2, 3):
                    for s in (0, 1):
                        nc.tensor.matmul(
                            py[:], attg[j][:, s, 128 * tb:128 * tb + 128],
                            wot_t[:, j, s, :], start=(n == 0), stop=(n == 3))
                        n += 1
                ysb = ysbp.tile([128, 512], f32, tag="ysb")
                nc.vector.tensor_add(ysb[:], py[:], ysbA[tb][:])
                nc.sync.dma_start(y_d[128 * tb:128 * tb + 128, :], ysb[:])

            def exchange_part(att_t, p_idx, lo, size):
                cols = slice(lo, lo + size)
                ci = dram.tile([128, size], bf16, tag=f"cci{p_idx}_{lo}",
                               name=f"cci{p_idx}_{lo}")
                co = dram.tile([2, 128, size], bf16, tag=f"cco{p_idx}_{lo}",
                               name=f"cco{p_idx}_{lo}")
                nc.sync.dma_start(ci[:], att_t[:, cols])
                if local_cc:
                    for s in range(2):
                        nc.gpsimd.dma_start(co[s], ci[:])
                else:
                    nc.gpsimd.collective_compute(
                        "AllGather", mybir.AluOpType.bypass,
                        replica_groups=[[0, 1], [2, 3], [4, 5], [6, 7]],
                        ins=[ci.opt()], outs=[co.opt()])
                for s in range(2):
                    nc.sync.dma_start(attg[p_idx][:, s, cols], co[s])

            # ---- attention unit stream ----
            units = [(qd, s2, g, cp)
                     for qd in (0, 1) for s2 in (0, 1)
                     for g in range(8) for cp in range(g + 1)]

            sc_of = {}

            def require(u):
                qd, s2, g, cp = u
                kq_piece(qd, "k", s2, cp)
                kq_piece(qd, "q", s2, g)
                v_piece(qd, 2 * cp)
                v_piece(qd, 2 * cp + 1)
                ones_piece(qd, 2 * cp)
                ones_piece(qd, 2 * cp + 1)

            def emit_qk(u):
                qd, s2, g, cp = u
                require(u)
                sc = psc.tile([128, 1024], f32, tag="sc")
                sc_of[u] = sc
                first = u == units[0]
                for hh in range(2):
                    pl = 64 * hh
                    for q2 in range(2):
                        c = 2 * cp + q2
                        # the wide exp reads the whole tile; the diagonal's
                        # skipped block holds stale-but-finite scores except
                        # on the very first unit (NaN-poisoned fresh PSUM),
                        # which therefore scores all 256 queries
                        qlo = 128 if (q2 == 1 and cp == g
                                      and not first) else 0
                        nc.tensor.matmul(
                            sc[:, 512 * hh + 256 * q2 + qlo:
                               512 * hh + 256 * q2 + 256],
                            ktq[qd][s2][pl:pl + 64, 128 * c:128 * c + 128],
                            qtq[qd][s2][pl:pl + 64,
                                        256 * g + qlo:256 * g + 256],
                            start=True, stop=True)

            emit_qk(units[0])
            po2 = None
            att_t = None
            pt_of = {}

            def emit_av(u):
                # runs one unit behind exp: its exp is already complete, so
                # these matmuls never stall the PE queue
                nonlocal po2, att_t
                qd, s2, g, cp = u
                p_idx = 2 * qd + s2
                if cp == 0 and g == 0:
                    att_t = sml.tile([128, _T], bf16, tag="att",
                                     name=f"att{p_idx}")
                if cp == 0:
                    po2 = [ppo.tile([128, 512], f32, tag=f"po{hh}",
                                    name=f"po{hh}")
                           for hh in range(2)]
                pt = pt_of.pop(u)
                for hh in range(2):
                    h_abs = 2 * s2 + hh
                    for q2 in range(2):
                        c = 2 * cp + q2
                        qlo = 128 if (q2 == 1 and cp == g) else 0
                        nc.tensor.matmul(
                            po2[hh][:, qlo:256],
                            va[qd][c][:, h_abs, :],
                            pt[:, 512 * hh + 256 * q2 + qlo:
                               512 * hh + 256 * q2 + 256],
                            start=(cp == 0 and q2 == 0),
                            stop=(cp == g and q2 == 1))

                if cp == g:
                    for hh in range(2):
                        # sums to a base-partition-0 tile (the reciprocal's
                        # custom-DVE op mis-executes on base-64 inputs);
                        # the AV lag gives po a full unit before reuse
                        pns = nrm.tile([64, 256], f32, tag="pns")
                        nc.vector.tensor_copy(pns[:], po2[hh][64:128, 0:256])
                        rec_t = nrm.tile([64, 256], f32, tag="rec")
                        nc.vector.reciprocal_approx_fast(rec_t[:], pns[:])
                        nc.vector.tensor_mul(
                            att_t[64 * hh:64 * hh + 64,
                                  256 * g:256 * g + 256],
                            po2[hh][0:64, 0:256], rec_t[:])
                        globals()['_dbg_pn_'] = pns
                        globals()['_dbg_rec_'] = rec_t
                    if g == 3:
                        exchange_part(att_t, p_idx, 0, 1024)
                        if p_idx == 3:
                            for tb in range(8):
                                bg.append(lambda tb=tb: stage2(tb))
                    if p_idx == 3:
                        if g == 5:
                            exchange_part(att_t, p_idx, 1024, 512)
                            for tb in range(8, 12):
                                bg.append(lambda tb=tb: stage2(tb))
                        if g == 6:
                            exchange_part(att_t, p_idx, 1536, 256)
                        if g == 7:
                            exchange_part(att_t, p_idx, 1792, 256)
                    elif g == 7:
                        exchange_part(att_t, p_idx, 1024, 1024)
                    if p_idx == 1 and g == 7:
                        for tb in range(16):
                            bg.append(lambda tb=tb: stage1(tb))

            for i, u in enumerate(units):
                qd, s2, g, cp = u
                sc = sc_of.pop(u)
                pt = ptp.tile([128, 1024], bf16, tag="pt")
                pt_of[u] = pt
                nc.scalar.activation(pt[:], sc[:], EXP, scale=_EXP_SCALE)

                if cp == g:
                    # diagonal: tri-mask blocks [0:128] and [384:512] per
                    # head with one strided mul each; block [256:384] is
                    # never read (AV's qlo trick)
                    for hh in range(2):
                        b = 512 * hh
                        quad_v = pt[:, b:b + 512].rearrange(
                            "p (blk m) -> p blk m", blk=4)[:, ::3, :]
                        nc.vector.tensor_mul(quad_v, quad_v, tri_b)

                if i + 1 < len(units):
                    emit_qk(units[i + 1])
                # spread remaining bg evenly over remaining units so the
                # PE never goes act-paced before the stream ends
                left = len(units) - i
                pump(max(1, (len(bg) + left - 1) // left))
                if i > 0:
                    emit_av(units[i - 1])

            emit_av(units[-1])
            while bg:
                pump(1)
            for tb in (12, 13, 14, 15):
                stage2(tb)

            if debug:
                def dcopy(nm, ap):
                    nc.gpsimd.dma_start(dbg_d[nm], ap)
                dcopy("kt", ktq[1][1][:])
                dcopy("qt", qtq[1][1][:])
                dcopy("va", va[1][15][:].rearrange("p h m -> p (h m)"))
                dcopy("attg", attg[3][:].rearrange("p s m -> p (s m)"))
                dcopy("ysbA", ysbA[0][:])
                dcopy("att", att_t[:])
                dcopy("pn", globals()['_dbg_pn_'][:])
                dcopy("rec", globals()['_dbg_rec_'][:])

    nc.compile()
    return nc


_NC_CACHE = {}


def _get_nc(repeats=1, local_cc=False, debug=False):
    key = (repeats, local_cc, debug)
    if key not in _NC_CACHE:
        _NC_CACHE[key] = _build_nc(repeats, local_cc, debug)
    return _NC_CACHE[key]


def _to_bf16(a):
    import ml_dtypes
    return np.asarray(a, dtype=ml_dtypes.bfloat16)


def _make_in_maps(x, wq, wk, wv, wo, bo):
    x = np.asarray(x, dtype=np.float32)
    wq = np.asarray(wq, dtype=np.float32)
    wk = np.asarray(wk, dtype=np.float32)
    wv = np.asarray(wv, dtype=np.float32)
    wo = np.asarray(wo, dtype=np.float32)
    bo = np.asarray(bo, dtype=np.float32)

    wq2 = wq.transpose(1, 0, 2).reshape(_E, _H * _D)  # [E, (16h, 64d)]
    wk2 = wk.transpose(1, 0, 2).reshape(_E, _H * _D)
    wv2 = wv.transpose(1, 0, 2).reshape(_E, _H * _D)
    wot = wo.T                                        # [hd, e_out]
    tri = np.triu(np.ones((128, 128), dtype=np.float32))

    def kq_layout(w2my):
        # [E, 512my] -> [128p, (qd, s2, e8, 2h, 64d)]
        a = w2my.reshape(8, 128, 2, 2, 2, 64)  # e, p, qd, s2, hh, d
        a = a.transpose(1, 2, 3, 0, 4, 5)      # p, qd, s2, e, hh, d
        return _to_bf16(np.ascontiguousarray(a.reshape(128, 4096)))

    def v_layout(w2my):
        # [E, 512my] -> [128p, (qd, e8, 4h x 64)]
        a = w2my.reshape(8, 128, 2, 256)       # e, p, qd, m
        a = a.transpose(1, 2, 0, 3)            # p, qd, e, m
        return _to_bf16(np.ascontiguousarray(a.reshape(128, 4096)))

    in_maps = []
    for c in range(_NCORES):
        b, h = c // 2, c % 2
        xt_h = np.ascontiguousarray(x[b].T)           # [E, T]
        xt_h = xt_h.reshape(8, 128, _T).transpose(1, 0, 2).reshape(128, -1)
        hs = slice(512 * h, 512 * h + 512)
        es = slice(_MYE * h, _MYE * h + _MYE)

        wot_my = wot[:, es]                           # [1024 hd, 512]
        w = np.empty((128, 4, 2, _MYE), dtype=np.float32)
        for j in range(4):
            for s in range(2):
                r0 = 64 * (8 * s + 2 * j)
                w[:, j, s, :] = wot_my[r0:r0 + 128, :]

        in_maps.append({
            "xt": _to_bf16(xt_h),
            "wk": kq_layout(wk2[:, hs]),
            "wq": kq_layout(wq2[:, hs]),
            "wv": v_layout(wv2[:, hs]),
            "wot": _to_bf16(w.reshape(128, 4096)),
            "bo_b": np.ascontiguousarray(
                np.broadcast_to(bo[es], (128, _MYE)).astype(np.float32)),
            "tri": _to_bf16(tri),
        })
    return in_maps


def kernel(x, wq, wk, wv, wo, bo, _want_results=False, _repeats=1, **_ignored):
    from concourse.bass_utils import run_bass_kernel_spmd

    nc = _get_nc(_repeats)
    in_maps = _make_in_maps(x, wq, wk, wv, wo, bo)
    res = run_bass_kernel_spmd(nc, in_maps, core_ids=list(range(_NCORES)))

    out = _assemble([res.results[c]["y"] for c in range(_NCORES)])
    if _want_results:
        return out, res
    return out


def _assemble(ys):
    out = np.empty((_B, _T, _E), dtype=np.float32)
    for c in range(_NCORES):
        b, h = c // 2, c % 2
        out[b][:, _MYE * h:_MYE * h + _MYE] = ys[c].reshape(_T, _MYE)
    return out
